# revision 36
# baseline (speedup 1.0000x reference)
"""GNN message-passing kernel (WeightedGNNConv x2) for 8 Trainium2 NeuronCores.

Sharding: edges are partitioned by dst-node range (12500 nodes per core), so
each core's segment-sums target disjoint node rows and no cross-core
reduction is needed.  Per core, edges are grouped into 98 windows of 128 dst
nodes; within a window they are grouped into 4 runs by src-range (the
dma_gather ANT instruction takes int16 indices, so the gather table is split
into 4 sub-tables of 25000 rows).  Each run is padded to a multiple of 128
"slots"; slot i of a run maps to SBUF partition i%128, tile i//128.  Pad
slots gather row 0 and carry zero edge-attrs, so their messages vanish.

Per window the device:
  1. streams one packed int16 plane [edge-attrs | gather idx | dst] (the
     attrs are host-permuted and 1/deg-prescaled; one DMA per window),
  2. dma_gathers x[src] (bf16) from the 4 sub-tables,
  3. multiplies them into bf16 messages,
  4. builds the one-hot scatter matrix sel[e, i, k] = (dst_rel[e,k] == i) on
     the vector engine in i-major layout — every operand's innermost axis is
     then stride-1, which keeps the DVE in its 2x perf mode (a k-major
     layout broadcasts dst along the innermost axis, stride 0, dropping the
     op to 1x and doubling its cost),
  5. accumulates aggT[c, n] += msg_tile[e, c].T @ sel_tile[e, n] on the
     tensor engine in PSUM (the segment-sum never touches HBM); the rhs
     reads sel with a strided access pattern,
  6. computes hT = relu(W0t.T @ xT + W0b.T @ aggT + b0) and DMAs it out.

The per-node mean (1/deg) is folded into the edge attributes on the host and
all node tensors are provided pre-transposed, so the device never divides or
transposes.  For layer 1 the gather table is bf16 h padded to 128 columns
(gather rows must be a multiple of 256 bytes) so the edge-attr multiply also
runs at 2x.  Two SPMD launches (layer 0, layer 1); the host gathers h
between them.
"""

import os
import time

import numpy as np

import concourse.bacc as bacc
import concourse.bass as bass
import concourse.mybir as mybir
import concourse.tile as tile
from concourse import library_config
from concourse.bass_utils import run_bass_kernel_spmd

N_NODES = 100000
N_EDGES = 1600000
DIN = 128
DH = 64
DOUT = 2
C = 8                      # cores
NCORE = N_NODES // C       # 12500 nodes per core
WIN = 128                  # dst nodes per window (sel/scatter cost scales
                           # with Fi*WIN; the packed per-window plane keeps
                           # the DMA-op count at one per window)
NWIN = (NCORE + WIN - 1) // WIN   # 98
NPAD = NWIN * WIN          # 12544 padded nodes per core
R = 4                      # src-range sub-tables
NSUB = N_NODES // R        # 25000 rows per sub-table

F32 = mybir.dt.float32
BF16 = mybir.dt.bfloat16
I16 = mybir.dt.int16

ENV_DT = mybir.dt.bfloat16  # edge-attr plane dtype (bf16 halves traffic)
ACT_DT = mybir.dt.bfloat16
MAX_GATHER_TILES = 8        # dma_gather crashes HW above 1024 indices/op

# pool depths (module-level so tests can bisect scheduling depth)
BUFS_STREAM = 4    # env/act, idx, dst, gat, sel pools
BUFS_NODE = 3      # xt/ht, agg, hw/ow, psum pools

_EXEC_TIMES_NS: list[int] = []


def _prep(x, edge_index, env_edge_attr, act_edge_attr):
    """Host-side sharding; see module docstring for the slot layout."""
    src = np.asarray(edge_index[0], dtype=np.int64)
    dst = np.asarray(edge_index[1], dtype=np.int64)
    E = src.shape[0]

    cnt = np.bincount(dst, minlength=N_NODES)
    s = (1.0 / np.maximum(cnt, 1.0)).astype(np.float32)

    core = dst // NCORE
    win = (dst % NCORE) // WIN                  # 0..NWIN-1
    rrng = src // NSUB                          # 0..R-1
    g = (core * NWIN + win) * R + rrng
    order = np.argsort(g, kind="stable")

    rcnt = np.bincount(g, minlength=C * NWIN * R).reshape(C, NWIN, R)
    Krun = -(-rcnt.max(axis=0) // 128)          # [NWIN, R] tiles per run
    Kwin = Krun.sum(axis=1)                     # [NWIN]
    troff = np.zeros((NWIN, R), np.int64)       # run tile offset within window
    troff[:, 1:] = np.cumsum(Krun[:, :-1], axis=1)
    offi = np.zeros(NWIN + 1, np.int64)         # window tile offsets
    np.cumsum(Kwin, out=offi[1:])
    Fi = int(offi[-1])                          # total tiles per core

    gsort = g[order]
    group_start = np.zeros(C * NWIN * R + 1, np.int64)
    np.cumsum(rcnt.ravel(), out=group_start[1:])
    j = np.arange(E) - group_start[gsort]       # rank within run
    cs = gsort // (NWIN * R)
    ws = (gsort // R) % NWIN
    rs = gsort % R
    t_ = offi[ws] + troff[ws, rs] + (j >> 7)    # tile
    p_ = j & 127                                # partition

    ids = np.full((C, Fi, 128), E, np.int64)
    ids[cs, t_, p_] = np.arange(E)

    def _plane(vals_sorted, pad, dt):
        v = np.concatenate([vals_sorted, np.full((1,) + vals_sorted.shape[1:],
                                                 pad, vals_sorted.dtype)])
        if v.ndim == 1:
            return np.ascontiguousarray(
                v[ids].transpose(0, 2, 1)).astype(dt, copy=False)
        D = v.shape[1]
        return np.ascontiguousarray(
            v[ids].transpose(0, 2, 1, 3)).reshape(C, 128, Fi * D).astype(
                dt, copy=False)

    worder = list(range(NWIN))

    # int16 gather indices in the wrapped layout: op slot i -> partition
    # i%16, column i//16, replicated across the 8 q7 partition groups.
    # Pad slots gather row 0 (their edge-attrs are 0, zeroing the message);
    # -1 skip-indices would need a per-core valid count in num_idxs_reg.
    src16 = np.concatenate([(src % NSUB).astype(np.int16)[order],
                            np.zeros(1, np.int16)])
    stream = src16[ids]                         # [C, Fi, 128] slot-ordered
    idx16 = np.tile(stream.reshape(C, Fi * 8, 16).transpose(0, 2, 1),
                    (1, 8, 1))                  # [C, 128, Fi*8]
    idx16 = np.ascontiguousarray(idx16)

    dst_rel = (dst - core * NCORE - win * WIN).astype(np.float32)
    dst_plane = _plane(dst_rel[order], -1.0, mybir.dt.np(BF16))

    se = s[dst][:, None]                        # fold mean 1/deg into attrs
    env2 = (np.asarray(env_edge_attr, np.float32) * se)[order]
    env_plane = _plane(env2, 0.0, mybir.dt.np(ENV_DT))
    del env2
    act2 = (np.asarray(act_edge_attr, np.float32) * se)[order]
    act_plane = _plane(act2, 0.0, mybir.dt.np(ACT_DT))
    del act2, ids

    def _pack(attr_plane, D):
        """One int16 plane per core with per-window blocks
        [attr Kw*D | idx Kw*8 | dst Kw] so each window needs one DMA."""
        P = D + 9
        pk = np.empty((C, 128, Fi * P), np.int16)
        for w in range(NWIN):
            o, Kw = offi[w], Kwin[w]
            blk = pk[:, :, o * P:(o + Kw) * P]
            blk[:, :, :Kw * D] = attr_plane[:, :, o * D:(o + Kw) * D].view(
                np.int16)
            blk[:, :, Kw * D:Kw * (D + 8)] = idx16[:, :, o * 8:(o + Kw) * 8]
            blk[:, :, Kw * (D + 8):] = dst_plane[:, :, o:o + Kw].view(np.int16)
        return np.ascontiguousarray(pk)

    pk0 = _pack(env_plane, DIN)
    pk1 = _pack(act_plane, DH)

    Kmax = int(Kwin.max())
    # i-major iota: iota2[p, i*Kmax + k] = i
    iota = np.repeat(np.arange(WIN, dtype=np.float32), Kmax)[None, :].repeat(
        128, 0).astype(mybir.dt.np(BF16))       # [128, WIN*Kmax]
    iota = np.ascontiguousarray(iota)

    xT = np.zeros((C, 128, NPAD), np.float32)
    x = np.asarray(x, np.float32)
    for c in range(C):
        xT[c, :, :NCORE] = x[c * NCORE:(c + 1) * NCORE].T

    return dict(Krun=Krun.tolist(), Kwin=Kwin.tolist(),
                troff=troff.tolist(), offi=offi.tolist(), Fi=Fi, Kmax=Kmax,
                worder=worder, pk0=pk0, pk1=pk1,
                iota=iota, xT=xT)


def _make_nc():
    # 4 SWDGE queues -> gather descriptor generation spreads over 4 q7
    # core pairs instead of 1 (per-op overhead was the prior bottleneck).
    return bacc.Bacc("TRN2", target_bir_lowering=False, debug=False,
                     num_swdge_queues=4)


def _emit_window_gathers(nc, gat_t, tabs, pk_t, idx_off, Krun_w, D):
    """Per src-range dma_gather ops filling gat_t's tiles, chunked to at
    most MAX_GATHER_TILES tiles (the HW op crashes above 1024 indices).
    Indices live in the packed plane tile at element offset idx_off."""
    a = 0       # tile offset within the window
    q = 0
    for r in range(R):
        Kr = Krun_w[r]
        while Kr > 0:
            kc = min(Kr, MAX_GATHER_TILES)
            n = kc * 128
            nc.gpsimd.dma_gather(
                gat_t[:, a * D:(a + kc) * D].rearrange(
                    "p (k d) -> p k d", d=D),
                tabs[r],
                pk_t[:, idx_off + a * 8:idx_off + a * 8 + n // 16],
                n, n, D,
                queue_num=q % 4)
            q += 1
            a += kc
            Kr -= kc


def _emit_sel(nc, sel_t, iota_res, dst_ap, Kw, Kmax):
    """One-hot scatter matrix in i-major layout (all innermost axes
    stride-1 so the DVE picks its 2x mode)."""
    nc.vector.tensor_tensor(
        out=sel_t[:].rearrange("p (i k) -> p i k", k=Kw),
        in0=iota_res[:].rearrange("p (i k) -> p i k", k=Kmax)[:, :, :Kw],
        in1=dst_ap.unsqueeze(1).broadcast_to([128, WIN, Kw]),
        op=mybir.AluOpType.is_equal,
    )


def build_l0(nc, p):
    """Layer 0: hT[64, NPAD] = relu(W0t.T @ xT + W0b.T @ aggT + b0)."""
    Krun, Kwin, offi, Fi, Kmax = (p["Krun"], p["Kwin"], p["offi"], p["Fi"],
                                  p["Kmax"])
    P0 = DIN + 9
    xg = nc.dram_tensor("xg", [N_NODES, DIN], BF16, kind="ExternalInput")
    xT = nc.dram_tensor("xT", [128, NPAD], F32, kind="ExternalInput")
    pk0 = nc.dram_tensor("pk0", [128, Fi * P0], I16, kind="ExternalInput")
    iotap = nc.dram_tensor("iotap", [128, WIN * Kmax], BF16,
                           kind="ExternalInput")
    w0t = nc.dram_tensor("w0t", [DIN, DH], F32, kind="ExternalInput")
    w0b = nc.dram_tensor("w0b", [DIN, DH], F32, kind="ExternalInput")
    b0 = nc.dram_tensor("b0", [DH, 1], F32, kind="ExternalInput")
    hT = nc.dram_tensor("hT", [DH, NPAD], F32, kind="ExternalOutput")
    tabs = [xg[r * NSUB:(r + 1) * NSUB, :] for r in range(R)]

    with tile.TileContext(nc) as tc:
        with (
            tc.tile_pool(name="const", bufs=1) as constp,
            tc.tile_pool(name="pk", bufs=BUFS_STREAM) as pk_pool,
            tc.tile_pool(name="gat", bufs=BUFS_STREAM) as gat_pool,
            tc.tile_pool(name="sel", bufs=BUFS_STREAM) as sel_pool,
            tc.tile_pool(name="xt", bufs=BUFS_NODE) as xt_pool,
            tc.tile_pool(name="agg", bufs=BUFS_NODE) as agg_pool,
            tc.tile_pool(name="hw", bufs=BUFS_NODE) as hw_pool,
            tc.tile_pool(name="pagg", bufs=BUFS_NODE, space="PSUM") as pagg_pool,
            tc.tile_pool(name="ph", bufs=BUFS_NODE, space="PSUM") as ph_pool,
        ):
            nc.gpsimd.load_library(library_config.mlp)
            iota_res = constp.tile([128, WIN * Kmax], BF16)
            w0t_res = constp.tile([DIN, DH], F32)
            w0b_res = constp.tile([DIN, DH], F32)
            b0_res = constp.tile([DH, 1], F32)
            nc.sync.dma_start(out=iota_res[:], in_=iotap[:])
            nc.scalar.dma_start(out=w0t_res[:], in_=w0t[:])
            nc.scalar.dma_start(out=w0b_res[:], in_=w0b[:])
            nc.scalar.dma_start(out=b0_res[:], in_=b0[:])

            for w in p["worder"]:
                Kw = Kwin[w]
                o = offi[w]
                pk_t = pk_pool.tile([128, Kw * P0], I16, tag="pk")
                nc.sync.dma_start(
                    out=pk_t[:], in_=pk0[:, o * P0:(o + Kw) * P0])
                env_t = pk_t[:, :Kw * DIN].bitcast(ENV_DT)
                dst_t = pk_t[:, Kw * (DIN + 8):].bitcast(BF16)

                gat_t = gat_pool.tile([128, Kw * DIN], BF16, tag="gat")
                _emit_window_gathers(nc, gat_t, tabs, pk_t, Kw * DIN,
                                     Krun[w], DIN)
                nc.vector.tensor_mul(gat_t[:], gat_t[:], env_t)
                sel_t = sel_pool.tile([128, Kw * WIN], BF16, tag="sel")
                _emit_sel(nc, sel_t, iota_res, dst_t, Kw, Kmax)
                selv = sel_t[:].rearrange("p (i k) -> p k i", k=Kw)
                pagg = pagg_pool.tile([128, WIN], F32)
                for k in range(Kw):
                    nc.tensor.matmul(
                        out=pagg[:],
                        lhsT=gat_t[:, k * DIN:(k + 1) * DIN],
                        rhs=selv[:, k],
                        start=(k == 0),
                        stop=(k == Kw - 1),
                    )
                agg_t = agg_pool.tile([128, WIN], F32, tag="agg")
                nc.scalar.copy(agg_t[:], pagg[:])

                xt_t = xt_pool.tile([128, WIN], F32, tag="xt")
                nc.scalar.dma_start(out=xt_t[:],
                                    in_=xT[:, w * WIN:(w + 1) * WIN])
                ph = ph_pool.tile([DH, WIN], F32)
                nc.tensor.matmul(out=ph[:], lhsT=w0t_res[:], rhs=xt_t[:],
                                 start=True, stop=False)
                nc.tensor.matmul(out=ph[:], lhsT=w0b_res[:], rhs=agg_t[:],
                                 start=False, stop=True)
                hw_t = hw_pool.tile([DH, WIN], F32, tag="hw")
                nc.scalar.activation(
                    out=hw_t[:], in_=ph[:],
                    func=mybir.ActivationFunctionType.Relu,
                    bias=b0_res[:, :1])
                nc.scalar.dma_start(out=hT[:, w * WIN:(w + 1) * WIN],
                                    in_=hw_t[:])
    nc.compile()
    return nc


def build_l1(nc, p):
    """Layer 1: outT[2, NPAD] = W1t.T @ hT + W1b.T @ agg1T + b1."""
    Krun, Kwin, offi, Fi, Kmax = (p["Krun"], p["Kwin"], p["offi"], p["Fi"],
                                  p["Kmax"])
    # gather table: bf16 h padded to 128 cols (gather rows must be 256B)
    P1 = DH + 9
    hg = nc.dram_tensor("hg", [N_NODES, DIN], BF16, kind="ExternalInput")
    hTp = nc.dram_tensor("hTp", [DH, NPAD], F32, kind="ExternalInput")
    pk1 = nc.dram_tensor("pk1", [128, Fi * P1], I16, kind="ExternalInput")
    iotap = nc.dram_tensor("iotap", [128, WIN * Kmax], BF16,
                           kind="ExternalInput")
    w1t = nc.dram_tensor("w1t", [DH, DOUT], F32, kind="ExternalInput")
    w1b = nc.dram_tensor("w1b", [DH, DOUT], F32, kind="ExternalInput")
    b1 = nc.dram_tensor("b1", [DOUT, 1], F32, kind="ExternalInput")
    outT = nc.dram_tensor("outT", [DOUT, NPAD], F32, kind="ExternalOutput")
    tabs = [hg[r * NSUB:(r + 1) * NSUB, :] for r in range(R)]

    with tile.TileContext(nc) as tc:
        with (
            tc.tile_pool(name="const", bufs=1) as constp,
            tc.tile_pool(name="pk", bufs=BUFS_STREAM) as pk_pool,
            tc.tile_pool(name="gat", bufs=BUFS_STREAM) as gat_pool,
            tc.tile_pool(name="msg", bufs=BUFS_STREAM) as msg_pool,
            tc.tile_pool(name="sel", bufs=BUFS_STREAM) as sel_pool,
            tc.tile_pool(name="ht", bufs=BUFS_NODE) as ht_pool,
            tc.tile_pool(name="agg", bufs=BUFS_NODE) as agg_pool,
            tc.tile_pool(name="ow", bufs=BUFS_NODE) as ow_pool,
            tc.tile_pool(name="pagg", bufs=BUFS_NODE, space="PSUM") as pagg_pool,
            tc.tile_pool(name="po", bufs=BUFS_NODE, space="PSUM") as po_pool,
        ):
            nc.gpsimd.load_library(library_config.mlp)
            iota_res = constp.tile([128, WIN * Kmax], BF16)
            w1t_res = constp.tile([DH, DOUT], F32)
            w1b_res = constp.tile([DH, DOUT], F32)
            b1_res = constp.tile([DOUT, 1], F32)
            nc.sync.dma_start(out=iota_res[:], in_=iotap[:])
            nc.scalar.dma_start(out=w1t_res[:], in_=w1t[:])
            nc.scalar.dma_start(out=w1b_res[:], in_=w1b[:])
            nc.scalar.dma_start(out=b1_res[:], in_=b1[:])

            for w in p["worder"]:
                Kw = Kwin[w]
                o = offi[w]
                pk_t = pk_pool.tile([128, Kw * P1], I16, tag="pk")
                nc.sync.dma_start(
                    out=pk_t[:], in_=pk1[:, o * P1:(o + Kw) * P1])
                act_t = pk_t[:, :Kw * DH].bitcast(ACT_DT)
                dst_t = pk_t[:, Kw * (DH + 8):].bitcast(BF16)

                gat_t = gat_pool.tile([128, Kw * DIN], BF16, tag="gat")
                _emit_window_gathers(nc, gat_t, tabs, pk_t, Kw * DH,
                                     Krun[w], DIN)
                msgb = msg_pool.tile([128, Kw * DH], BF16, tag="msgb")
                nc.vector.tensor_mul(
                    msgb[:].rearrange("p (k d) -> p k d", d=DH),
                    gat_t[:].rearrange("p (k d) -> p k d", d=DIN)[:, :, :DH],
                    act_t.rearrange("p (k d) -> p k d", d=DH),
                )
                sel_t = sel_pool.tile([128, Kw * WIN], BF16, tag="sel")
                _emit_sel(nc, sel_t, iota_res, dst_t, Kw, Kmax)
                selv = sel_t[:].rearrange("p (i k) -> p k i", k=Kw)
                pagg = pagg_pool.tile([DH, WIN], F32)
                for k in range(Kw):
                    nc.tensor.matmul(
                        out=pagg[:],
                        lhsT=msgb[:, k * DH:(k + 1) * DH],
                        rhs=selv[:, k],
                        start=(k == 0),
                        stop=(k == Kw - 1),
                    )
                agg_t = agg_pool.tile([DH, WIN], F32, tag="agg")
                nc.scalar.copy(agg_t[:], pagg[:])

                ht_t = ht_pool.tile([DH, WIN], F32, tag="ht")
                nc.scalar.dma_start(out=ht_t[:],
                                    in_=hTp[:, w * WIN:(w + 1) * WIN])
                po = po_pool.tile([DOUT, WIN], F32)
                nc.tensor.matmul(out=po[:], lhsT=w1t_res[:], rhs=ht_t[:],
                                 start=True, stop=False)
                nc.tensor.matmul(out=po[:], lhsT=w1b_res[:], rhs=agg_t[:],
                                 start=False, stop=True)
                ow_t = ow_pool.tile([DOUT, WIN], F32, tag="ow")
                nc.scalar.add(out=ow_t[:], in_=po[:], add=b1_res[:, :1])
                nc.scalar.dma_start(out=outT[:, w * WIN:(w + 1) * WIN],
                                    in_=ow_t[:])
    nc.compile()
    return nc


def _time_spmd(nc, in_maps, reps, label):
    """Wall-clock the compiled SPMD executable with device-resident inputs.

    The axon NTFF profile hook isn't available in this container, so HW exec
    time is estimated as (T(reps) - T(1)) / (reps - 1) over asynchronously
    dispatched back-to-back executions — pipelining cancels the tunnel RTT.
    """
    import jax
    from jax.sharding import Mesh, PartitionSpec, NamedSharding
    from jax.experimental.shard_map import shard_map
    from concourse import bass2jax, mybir as mb

    bass2jax.install_neuronx_cc_hook()
    part_name = nc.partition_id_tensor.name if nc.partition_id_tensor else None
    in_names, out_names, out_avals, zero_outs = [], [], [], []
    for alloc in nc.m.functions[0].allocations:
        if not isinstance(alloc, mb.MemoryLocationSet):
            continue
        name = alloc.memorylocations[0].name
        if alloc.kind == "ExternalInput":
            if name != part_name:
                in_names.append(name)
        elif alloc.kind == "ExternalOutput":
            out_names.append(name)
            shape = tuple(alloc.tensor_shape)
            dtype = mb.dt.np(alloc.dtype)
            out_avals.append(jax.core.ShapedArray(shape, dtype))
            zero_outs.append(np.zeros(shape, dtype))
    n_params = len(in_names)
    all_names = in_names + out_names
    if part_name is not None:
        all_names = all_names + [part_name]

    def _call(*args):
        operands = list(args)
        if part_name is not None:
            operands.append(bass2jax.partition_id_tensor())
        outs = bass2jax._bass_exec_p.bind(
            *operands,
            out_avals=tuple(out_avals),
            in_names=tuple(all_names),
            out_names=tuple(out_names),
            lowering_input_output_aliases=(),
            sim_require_finite=True,
            sim_require_nnan=True,
            nc=nc,
        )
        return tuple(outs)

    devices = jax.devices()[:C]
    mesh = Mesh(np.asarray(devices), ("core",))
    nouts = len(out_names)
    f = jax.jit(
        shard_map(_call, mesh=mesh,
                  in_specs=(PartitionSpec("core"),) * (n_params + nouts),
                  out_specs=(PartitionSpec("core"),) * nouts,
                  check_rep=False),
        keep_unused=True,
    )
    sh = NamedSharding(mesh, PartitionSpec("core"))
    args = [
        jax.device_put(
            np.concatenate([np.asarray(m[name]) for m in in_maps], axis=0), sh)
        for name in in_names
    ] + [
        jax.device_put(
            np.zeros((C * z.shape[0], *z.shape[1:]), z.dtype), sh)
        for z in zero_outs
    ]

    def timed(k):
        # k async back-to-back dispatches; the terminal pipelines them, so
        # the k-slope isolates device execution from tunnel RTT.
        t0 = time.time()
        rs = [f(*args) for _ in range(k)]
        jax.block_until_ready(rs)
        return time.time() - t0

    timed(1)                            # compile + warmup
    timed(reps)
    # The tunnel adds bursty positive noise per dispatch, so the device
    # exec time is the slope of the best-case envelope: min T(reps) over
    # many interleaved samples against the median T(1) dispatch floor (the
    # min T(1) occasionally under-measures and would inflate the slope).
    # In congested windows the whole batch pipelines inside one inflated
    # round trip and the slope degenerates toward 0; detect that (tiny
    # slope or inflated floor) and resample, keeping the largest estimate
    # as a fallback.
    nsamp = 14
    best = 0
    floor0 = None
    for attempt in range(4):
        t1s, tns = [], []
        for _ in range(nsamp):
            t1s.append(timed(1))
            tns.append(timed(reps))
        t1s.sort()
        tns.sort()
        med1, minn = t1s[len(t1s) // 2], tns[0]
        floor0 = min(floor0, t1s[0]) if floor0 is not None else t1s[0]
        est = int(max(minn - min(med1, minn), 0.0) / (reps - 1) * 1e9)
        best = max(best, est)
        clean = med1 <= 1.1 * floor0 and est >= 30_000
        print(f"[kernel] {label}[{attempt}]: medT(1)={med1*1e3:.2f} ms"
              f"  minT({reps})={minn*1e3:.2f} ms  est={est} ns"
              f"{' ok' if clean else ' retry'}", flush=True)
        if clean:
            return est
    return best


def _run(nc, in_maps, label):
    res = run_bass_kernel_spmd(nc, in_maps, list(range(C)))
    reps = int(os.environ.get("GNN_TIME_REPS", "0"))
    if reps > 1:
        _EXEC_TIMES_NS.append(_time_spmd(nc, in_maps, reps, label))
    return res.results


def kernel(x, edge_index, env_edge_attr, act_edge_attr, W0, b0, W1, b1):
    _EXEC_TIMES_NS.clear()

    x = np.asarray(x, np.float32)
    p = _prep(x, edge_index, env_edge_attr, act_edge_attr)

    xg = np.ascontiguousarray(x.astype(mybir.dt.np(BF16)))
    w0t = np.ascontiguousarray(np.asarray(W0, np.float32)[:DIN])
    w0b = np.ascontiguousarray(np.asarray(W0, np.float32)[DIN:])
    b0v = np.asarray(b0, np.float32).reshape(DH, 1)
    w1t = np.ascontiguousarray(np.asarray(W1, np.float32)[:DH])
    w1b = np.ascontiguousarray(np.asarray(W1, np.float32)[DH:])
    b1v = np.asarray(b1, np.float32).reshape(DOUT, 1)

    # ---- layer 0 ----
    nc0 = build_l0(_make_nc(), p)
    in_maps0 = [
        dict(xg=xg, xT=p["xT"][c], pk0=p["pk0"][c],
             iotap=p["iota"], w0t=w0t, w0b=w0b, b0=b0v)
        for c in range(C)
    ]
    res0 = _run(nc0, in_maps0, "L0")

    h = np.empty((N_NODES, DH), np.float32)
    hT_all = np.empty((C, DH, NPAD), np.float32)
    for c in range(C):
        hT_all[c] = res0[c]["hT"]
        h[c * NCORE:(c + 1) * NCORE] = hT_all[c][:, :NCORE].T
    hgb = np.zeros((N_NODES, DIN), mybir.dt.np(BF16))
    hgb[:, :DH] = h

    # ---- layer 1 ----
    nc1 = build_l1(_make_nc(), p)
    in_maps1 = [
        dict(hg=hgb, hTp=hT_all[c], pk1=p["pk1"][c],
             iotap=p["iota"], w1t=w1t, w1b=w1b, b1=b1v)
        for c in range(C)
    ]
    res1 = _run(nc1, in_maps1, "L1")

    out = np.empty((N_NODES, DOUT), np.float32)
    for c in range(C):
        out[c * NCORE:(c + 1) * NCORE] = res1[c]["outT"][:, :NCORE].T
    if _EXEC_TIMES_NS:
        print(f"[kernel] total HW exec time: {sum(_EXEC_TIMES_NS)} ns",
              flush=True)
    return out


# revision 38
# speedup vs baseline: 1.7271x; 1.7271x over previous
"""GNN message-passing kernel (WeightedGNNConv x2) for 8 Trainium2 NeuronCores.

Sharding: edges are partitioned by dst-node range (12500 nodes per core), so
each core's segment-sums target disjoint node rows and no cross-core
reduction is needed.  Per core, edges are grouped into 98 windows of 128 dst
nodes; within a window they are grouped into 4 runs by src-range (the
dma_gather ANT instruction takes int16 indices, so the gather table is split
into 4 sub-tables of 25000 rows).  Each run is padded to a multiple of 128
"slots"; slot i of a run maps to SBUF partition i%128, tile i//128.  Pad
slots gather row 0 and carry zero edge-attrs, so their messages vanish.

Per window the device:
  1. streams one packed int16 plane [edge-attrs | gather idx | dst] (the
     attrs are host-permuted and 1/deg-prescaled; one DMA per window),
  2. dma_gathers x[src] (bf16) from the 4 sub-tables,
  3. multiplies them into bf16 messages,
  4. builds the one-hot scatter matrix sel[e, i, k] = (dst_rel[e,k] == i) on
     the vector engine in i-major layout — every operand's innermost axis is
     then stride-1, which keeps the DVE in its 2x perf mode (a k-major
     layout broadcasts dst along the innermost axis, stride 0, dropping the
     op to 1x and doubling its cost),
  5. accumulates aggT[c, n] += msg_tile[e, c].T @ sel_tile[e, n] on the
     tensor engine in PSUM (the segment-sum never touches HBM); the rhs
     reads sel with a strided access pattern,
  6. computes hT = relu(W0t.T @ xT + W0b.T @ aggT + b0) and DMAs it out.

The per-node mean (1/deg) is folded into the edge attributes on the host and
all node tensors are provided pre-transposed, so the device never divides or
transposes.  For layer 1 the gather table is bf16 h padded to 128 columns
(gather rows must be a multiple of 256 bytes) so the edge-attr multiply also
runs at 2x.  Two SPMD launches (layer 0, layer 1); the host gathers h
between them.
"""

import os
import time

import numpy as np

import concourse.bacc as bacc
import concourse.bass as bass
import concourse.mybir as mybir
import concourse.tile as tile
from concourse import library_config
from concourse.bass_utils import run_bass_kernel_spmd

N_NODES = 100000
N_EDGES = 1600000
DIN = 128
DH = 64
DOUT = 2
C = 8                      # cores
NCORE = N_NODES // C       # 12500 nodes per core
WIN = 128                  # dst nodes per window (sel/scatter cost scales
                           # with Fi*WIN; the packed per-window plane keeps
                           # the DMA-op count at one per window)
NWIN = (NCORE + WIN - 1) // WIN   # 98
NPAD = NWIN * WIN          # 12544 padded nodes per core
R = 4                      # src-range sub-tables
NSUB = N_NODES // R        # 25000 rows per sub-table

F32 = mybir.dt.float32
BF16 = mybir.dt.bfloat16
I16 = mybir.dt.int16

ENV_DT = mybir.dt.bfloat16  # edge-attr plane dtype (bf16 halves traffic)
ACT_DT = mybir.dt.bfloat16
MAX_GATHER_TILES = 8        # dma_gather crashes HW above 1024 indices/op

# pool depths (module-level so tests can bisect scheduling depth)
BUFS_STREAM = 4    # env/act, idx, dst, gat, sel pools
BUFS_NODE = 3      # xt/ht, agg, hw/ow, psum pools

_EXEC_TIMES_NS: list[int] = []


def _prep(x, edge_index, env_edge_attr, act_edge_attr):
    """Host-side sharding; see module docstring for the slot layout."""
    src = np.asarray(edge_index[0], dtype=np.int64)
    dst = np.asarray(edge_index[1], dtype=np.int64)
    E = src.shape[0]

    cnt = np.bincount(dst, minlength=N_NODES)
    s = (1.0 / np.maximum(cnt, 1.0)).astype(np.float32)

    core = dst // NCORE
    win = (dst % NCORE) // WIN                  # 0..NWIN-1
    rrng = src // NSUB                          # 0..R-1
    g = (core * NWIN + win) * R + rrng
    order = np.argsort(g, kind="stable")

    rcnt = np.bincount(g, minlength=C * NWIN * R).reshape(C, NWIN, R)
    Krun = -(-rcnt.max(axis=0) // 128)          # [NWIN, R] tiles per run
    Kwin = Krun.sum(axis=1)                     # [NWIN]
    troff = np.zeros((NWIN, R), np.int64)       # run tile offset within window
    troff[:, 1:] = np.cumsum(Krun[:, :-1], axis=1)
    offi = np.zeros(NWIN + 1, np.int64)         # window tile offsets
    np.cumsum(Kwin, out=offi[1:])
    Fi = int(offi[-1])                          # total tiles per core

    gsort = g[order]
    group_start = np.zeros(C * NWIN * R + 1, np.int64)
    np.cumsum(rcnt.ravel(), out=group_start[1:])
    j = np.arange(E) - group_start[gsort]       # rank within run
    cs = gsort // (NWIN * R)
    ws = (gsort // R) % NWIN
    rs = gsort % R
    t_ = offi[ws] + troff[ws, rs] + (j >> 7)    # tile
    p_ = j & 127                                # partition

    ids = np.full((C, Fi, 128), E, np.int64)
    ids[cs, t_, p_] = np.arange(E)

    def _plane(vals_sorted, pad, dt):
        v = np.concatenate([vals_sorted, np.full((1,) + vals_sorted.shape[1:],
                                                 pad, vals_sorted.dtype)])
        if v.ndim == 1:
            return np.ascontiguousarray(
                v[ids].transpose(0, 2, 1)).astype(dt, copy=False)
        D = v.shape[1]
        return np.ascontiguousarray(
            v[ids].transpose(0, 2, 1, 3)).reshape(C, 128, Fi * D).astype(
                dt, copy=False)

    worder = list(range(NWIN))

    # int16 gather indices in the wrapped layout: op slot i -> partition
    # i%16, column i//16, replicated across the 8 q7 partition groups.
    # Pad slots gather row 0 (their edge-attrs are 0, zeroing the message);
    # -1 skip-indices would need a per-core valid count in num_idxs_reg.
    src16 = np.concatenate([(src % NSUB).astype(np.int16)[order],
                            np.zeros(1, np.int16)])
    stream = src16[ids]                         # [C, Fi, 128] slot-ordered
    idx16 = np.tile(stream.reshape(C, Fi * 8, 16).transpose(0, 2, 1),
                    (1, 8, 1))                  # [C, 128, Fi*8]
    idx16 = np.ascontiguousarray(idx16)

    dst_rel = (dst - core * NCORE - win * WIN).astype(np.float32)
    dst_plane = _plane(dst_rel[order], -1.0, mybir.dt.np(BF16))

    se = s[dst][:, None]                        # fold mean 1/deg into attrs
    env2 = (np.asarray(env_edge_attr, np.float32) * se)[order]
    env_plane = _plane(env2, 0.0, mybir.dt.np(ENV_DT))
    del env2
    act2 = (np.asarray(act_edge_attr, np.float32) * se)[order]
    act_plane = _plane(act2, 0.0, mybir.dt.np(ACT_DT))
    del act2, ids

    def _pack(attr_plane, D):
        """One int16 plane per core with per-window blocks
        [attr Kw*D | idx Kw*8 | dst Kw] so each window needs one DMA."""
        P = D + 9
        pk = np.empty((C, 128, Fi * P), np.int16)
        for w in range(NWIN):
            o, Kw = offi[w], Kwin[w]
            blk = pk[:, :, o * P:(o + Kw) * P]
            blk[:, :, :Kw * D] = attr_plane[:, :, o * D:(o + Kw) * D].view(
                np.int16)
            blk[:, :, Kw * D:Kw * (D + 8)] = idx16[:, :, o * 8:(o + Kw) * 8]
            blk[:, :, Kw * (D + 8):] = dst_plane[:, :, o:o + Kw].view(np.int16)
        return np.ascontiguousarray(pk)

    pk0 = _pack(env_plane, DIN)
    pk1 = _pack(act_plane, DH)

    Kmax = int(Kwin.max())
    # i-major iota: iota2[p, i*Kmax + k] = i
    iota = np.repeat(np.arange(WIN, dtype=np.float32), Kmax)[None, :].repeat(
        128, 0).astype(mybir.dt.np(BF16))       # [128, WIN*Kmax]
    iota = np.ascontiguousarray(iota)

    xT = np.zeros((C, 128, NPAD), np.float32)
    x = np.asarray(x, np.float32)
    for c in range(C):
        xT[c, :, :NCORE] = x[c * NCORE:(c + 1) * NCORE].T

    return dict(Krun=Krun.tolist(), Kwin=Kwin.tolist(),
                troff=troff.tolist(), offi=offi.tolist(), Fi=Fi, Kmax=Kmax,
                worder=worder, pk0=pk0, pk1=pk1,
                iota=iota, xT=xT)


def _make_nc():
    # 4 SWDGE queues -> gather descriptor generation spreads over 4 q7
    # core pairs instead of 1 (per-op overhead was the prior bottleneck).
    return bacc.Bacc("TRN2", target_bir_lowering=False, debug=False,
                     num_swdge_queues=4)


def _emit_window_gathers(nc, gat_t, tabs, pk_t, idx_off, Krun_w, D):
    """Per src-range dma_gather ops filling gat_t's tiles, chunked to at
    most MAX_GATHER_TILES tiles (the HW op crashes above 1024 indices).
    Indices live in the packed plane tile at element offset idx_off."""
    a = 0       # tile offset within the window
    q = 0
    for r in range(R):
        Kr = Krun_w[r]
        while Kr > 0:
            kc = min(Kr, MAX_GATHER_TILES)
            n = kc * 128
            nc.gpsimd.dma_gather(
                gat_t[:, a * D:(a + kc) * D].rearrange(
                    "p (k d) -> p k d", d=D),
                tabs[r],
                pk_t[:, idx_off + a * 8:idx_off + a * 8 + n // 16],
                n, n, D,
                queue_num=q % 4)
            q += 1
            a += kc
            Kr -= kc


def _emit_sel(nc, sel_t, iota_res, dst_ap, Kw, Kmax):
    """One-hot scatter matrix in i-major layout (all innermost axes
    stride-1 so the DVE picks its 2x mode)."""
    nc.vector.tensor_tensor(
        out=sel_t[:].rearrange("p (i k) -> p i k", k=Kw),
        in0=iota_res[:].rearrange("p (i k) -> p i k", k=Kmax)[:, :, :Kw],
        in1=dst_ap.unsqueeze(1).broadcast_to([128, WIN, Kw]),
        op=mybir.AluOpType.is_equal,
    )


def build_l0(nc, p):
    """Layer 0: hT[64, NPAD] = relu(W0t.T @ xT + W0b.T @ aggT + b0)."""
    Krun, Kwin, offi, Fi, Kmax = (p["Krun"], p["Kwin"], p["offi"], p["Fi"],
                                  p["Kmax"])
    P0 = DIN + 9
    xg = nc.dram_tensor("xg", [N_NODES, DIN], BF16, kind="ExternalInput")
    xT = nc.dram_tensor("xT", [128, NPAD], F32, kind="ExternalInput")
    pk0 = nc.dram_tensor("pk0", [128, Fi * P0], I16, kind="ExternalInput")
    iotap = nc.dram_tensor("iotap", [128, WIN * Kmax], BF16,
                           kind="ExternalInput")
    w0t = nc.dram_tensor("w0t", [DIN, DH], F32, kind="ExternalInput")
    w0b = nc.dram_tensor("w0b", [DIN, DH], F32, kind="ExternalInput")
    b0 = nc.dram_tensor("b0", [DH, 1], F32, kind="ExternalInput")
    hT = nc.dram_tensor("hT", [DH, NPAD], F32, kind="ExternalOutput")
    tabs = [xg[r * NSUB:(r + 1) * NSUB, :] for r in range(R)]

    with tile.TileContext(nc) as tc:
        with (
            tc.tile_pool(name="const", bufs=1) as constp,
            tc.tile_pool(name="pk", bufs=BUFS_STREAM) as pk_pool,
            tc.tile_pool(name="gat", bufs=BUFS_STREAM) as gat_pool,
            tc.tile_pool(name="sel", bufs=BUFS_STREAM) as sel_pool,
            tc.tile_pool(name="xt", bufs=BUFS_NODE) as xt_pool,
            tc.tile_pool(name="agg", bufs=BUFS_NODE) as agg_pool,
            tc.tile_pool(name="hw", bufs=BUFS_NODE) as hw_pool,
            tc.tile_pool(name="pagg", bufs=BUFS_NODE, space="PSUM") as pagg_pool,
            tc.tile_pool(name="ph", bufs=BUFS_NODE, space="PSUM") as ph_pool,
        ):
            nc.gpsimd.load_library(library_config.mlp)
            iota_res = constp.tile([128, WIN * Kmax], BF16)
            w0t_res = constp.tile([DIN, DH], F32)
            w0b_res = constp.tile([DIN, DH], F32)
            b0_res = constp.tile([DH, 1], F32)
            nc.sync.dma_start(out=iota_res[:], in_=iotap[:])
            nc.scalar.dma_start(out=w0t_res[:], in_=w0t[:])
            nc.scalar.dma_start(out=w0b_res[:], in_=w0b[:])
            nc.scalar.dma_start(out=b0_res[:], in_=b0[:])

            for w in p["worder"]:
                Kw = Kwin[w]
                o = offi[w]
                pk_t = pk_pool.tile([128, Kw * P0], I16, tag="pk")
                nc.sync.dma_start(
                    out=pk_t[:], in_=pk0[:, o * P0:(o + Kw) * P0])
                env_t = pk_t[:, :Kw * DIN].bitcast(ENV_DT)
                dst_t = pk_t[:, Kw * (DIN + 8):].bitcast(BF16)

                gat_t = gat_pool.tile([128, Kw * DIN], BF16, tag="gat")
                _emit_window_gathers(nc, gat_t, tabs, pk_t, Kw * DIN,
                                     Krun[w], DIN)
                nc.vector.tensor_mul(gat_t[:], gat_t[:], env_t)
                sel_t = sel_pool.tile([128, Kw * WIN], BF16, tag="sel")
                _emit_sel(nc, sel_t, iota_res, dst_t, Kw, Kmax)
                selv = sel_t[:].rearrange("p (i k) -> p k i", k=Kw)
                pagg = pagg_pool.tile([128, WIN], F32)
                for k in range(Kw):
                    nc.tensor.matmul(
                        out=pagg[:],
                        lhsT=gat_t[:, k * DIN:(k + 1) * DIN],
                        rhs=selv[:, k],
                        start=(k == 0),
                        stop=(k == Kw - 1),
                    )
                agg_t = agg_pool.tile([128, WIN], F32, tag="agg")
                nc.scalar.copy(agg_t[:], pagg[:])

                xt_t = xt_pool.tile([128, WIN], F32, tag="xt")
                nc.scalar.dma_start(out=xt_t[:],
                                    in_=xT[:, w * WIN:(w + 1) * WIN])
                ph = ph_pool.tile([DH, WIN], F32)
                nc.tensor.matmul(out=ph[:], lhsT=w0t_res[:], rhs=xt_t[:],
                                 start=True, stop=False)
                nc.tensor.matmul(out=ph[:], lhsT=w0b_res[:], rhs=agg_t[:],
                                 start=False, stop=True)
                hw_t = hw_pool.tile([DH, WIN], F32, tag="hw")
                nc.scalar.activation(
                    out=hw_t[:], in_=ph[:],
                    func=mybir.ActivationFunctionType.Relu,
                    bias=b0_res[:, :1])
                nc.scalar.dma_start(out=hT[:, w * WIN:(w + 1) * WIN],
                                    in_=hw_t[:])
    nc.compile()
    return nc


def build_l1(nc, p):
    """Layer 1: outT[2, NPAD] = W1t.T @ hT + W1b.T @ agg1T + b1."""
    Krun, Kwin, offi, Fi, Kmax = (p["Krun"], p["Kwin"], p["offi"], p["Fi"],
                                  p["Kmax"])
    # gather table: bf16 h padded to 128 cols (gather rows must be 256B)
    P1 = DH + 9
    hg = nc.dram_tensor("hg", [N_NODES, DIN], BF16, kind="ExternalInput")
    hTp = nc.dram_tensor("hTp", [DH, NPAD], F32, kind="ExternalInput")
    pk1 = nc.dram_tensor("pk1", [128, Fi * P1], I16, kind="ExternalInput")
    iotap = nc.dram_tensor("iotap", [128, WIN * Kmax], BF16,
                           kind="ExternalInput")
    w1t = nc.dram_tensor("w1t", [DH, DOUT], F32, kind="ExternalInput")
    w1b = nc.dram_tensor("w1b", [DH, DOUT], F32, kind="ExternalInput")
    b1 = nc.dram_tensor("b1", [DOUT, 1], F32, kind="ExternalInput")
    outT = nc.dram_tensor("outT", [DOUT, NPAD], F32, kind="ExternalOutput")
    tabs = [hg[r * NSUB:(r + 1) * NSUB, :] for r in range(R)]

    with tile.TileContext(nc) as tc:
        with (
            tc.tile_pool(name="const", bufs=1) as constp,
            tc.tile_pool(name="pk", bufs=BUFS_STREAM) as pk_pool,
            tc.tile_pool(name="gat", bufs=BUFS_STREAM) as gat_pool,
            tc.tile_pool(name="msg", bufs=BUFS_STREAM) as msg_pool,
            tc.tile_pool(name="sel", bufs=BUFS_STREAM) as sel_pool,
            tc.tile_pool(name="ht", bufs=BUFS_NODE) as ht_pool,
            tc.tile_pool(name="agg", bufs=BUFS_NODE) as agg_pool,
            tc.tile_pool(name="ow", bufs=BUFS_NODE) as ow_pool,
            tc.tile_pool(name="pagg", bufs=BUFS_NODE, space="PSUM") as pagg_pool,
            tc.tile_pool(name="po", bufs=BUFS_NODE, space="PSUM") as po_pool,
        ):
            nc.gpsimd.load_library(library_config.mlp)
            iota_res = constp.tile([128, WIN * Kmax], BF16)
            w1t_res = constp.tile([DH, DOUT], F32)
            w1b_res = constp.tile([DH, DOUT], F32)
            b1_res = constp.tile([DOUT, 1], F32)
            nc.sync.dma_start(out=iota_res[:], in_=iotap[:])
            nc.scalar.dma_start(out=w1t_res[:], in_=w1t[:])
            nc.scalar.dma_start(out=w1b_res[:], in_=w1b[:])
            nc.scalar.dma_start(out=b1_res[:], in_=b1[:])

            for w in p["worder"]:
                Kw = Kwin[w]
                o = offi[w]
                pk_t = pk_pool.tile([128, Kw * P1], I16, tag="pk")
                nc.sync.dma_start(
                    out=pk_t[:], in_=pk1[:, o * P1:(o + Kw) * P1])
                act_t = pk_t[:, :Kw * DH].bitcast(ACT_DT)
                dst_t = pk_t[:, Kw * (DH + 8):].bitcast(BF16)

                gat_t = gat_pool.tile([128, Kw * DIN], BF16, tag="gat")
                _emit_window_gathers(nc, gat_t, tabs, pk_t, Kw * DH,
                                     Krun[w], DIN)
                msgb = msg_pool.tile([128, Kw * DH], BF16, tag="msgb")
                nc.vector.tensor_mul(
                    msgb[:].rearrange("p (k d) -> p k d", d=DH),
                    gat_t[:].rearrange("p (k d) -> p k d", d=DIN)[:, :, :DH],
                    act_t.rearrange("p (k d) -> p k d", d=DH),
                )
                sel_t = sel_pool.tile([128, Kw * WIN], BF16, tag="sel")
                _emit_sel(nc, sel_t, iota_res, dst_t, Kw, Kmax)
                selv = sel_t[:].rearrange("p (i k) -> p k i", k=Kw)
                pagg = pagg_pool.tile([DH, WIN], F32)
                for k in range(Kw):
                    nc.tensor.matmul(
                        out=pagg[:],
                        lhsT=msgb[:, k * DH:(k + 1) * DH],
                        rhs=selv[:, k],
                        start=(k == 0),
                        stop=(k == Kw - 1),
                    )
                agg_t = agg_pool.tile([DH, WIN], F32, tag="agg")
                nc.scalar.copy(agg_t[:], pagg[:])

                ht_t = ht_pool.tile([DH, WIN], F32, tag="ht")
                nc.scalar.dma_start(out=ht_t[:],
                                    in_=hTp[:, w * WIN:(w + 1) * WIN])
                po = po_pool.tile([DOUT, WIN], F32)
                nc.tensor.matmul(out=po[:], lhsT=w1t_res[:], rhs=ht_t[:],
                                 start=True, stop=False)
                nc.tensor.matmul(out=po[:], lhsT=w1b_res[:], rhs=agg_t[:],
                                 start=False, stop=True)
                ow_t = ow_pool.tile([DOUT, WIN], F32, tag="ow")
                nc.scalar.add(out=ow_t[:], in_=po[:], add=b1_res[:, :1])
                nc.scalar.dma_start(out=outT[:, w * WIN:(w + 1) * WIN],
                                    in_=ow_t[:])
    nc.compile()
    return nc


def _time_spmd(nc, in_maps, reps, label):
    """Wall-clock the compiled SPMD executable with device-resident inputs.

    The axon NTFF profile hook isn't available in this container, so HW exec
    time is estimated as (T(reps) - T(1)) / (reps - 1) over asynchronously
    dispatched back-to-back executions — pipelining cancels the tunnel RTT.
    """
    import jax
    from jax.sharding import Mesh, PartitionSpec, NamedSharding
    from jax.experimental.shard_map import shard_map
    from concourse import bass2jax, mybir as mb

    bass2jax.install_neuronx_cc_hook()
    part_name = nc.partition_id_tensor.name if nc.partition_id_tensor else None
    in_names, out_names, out_avals, zero_outs = [], [], [], []
    for alloc in nc.m.functions[0].allocations:
        if not isinstance(alloc, mb.MemoryLocationSet):
            continue
        name = alloc.memorylocations[0].name
        if alloc.kind == "ExternalInput":
            if name != part_name:
                in_names.append(name)
        elif alloc.kind == "ExternalOutput":
            out_names.append(name)
            shape = tuple(alloc.tensor_shape)
            dtype = mb.dt.np(alloc.dtype)
            out_avals.append(jax.core.ShapedArray(shape, dtype))
            zero_outs.append(np.zeros(shape, dtype))
    n_params = len(in_names)
    all_names = in_names + out_names
    if part_name is not None:
        all_names = all_names + [part_name]

    def _call(*args):
        operands = list(args)
        if part_name is not None:
            operands.append(bass2jax.partition_id_tensor())
        outs = bass2jax._bass_exec_p.bind(
            *operands,
            out_avals=tuple(out_avals),
            in_names=tuple(all_names),
            out_names=tuple(out_names),
            lowering_input_output_aliases=(),
            sim_require_finite=True,
            sim_require_nnan=True,
            nc=nc,
        )
        return tuple(outs)

    devices = jax.devices()[:C]
    mesh = Mesh(np.asarray(devices), ("core",))
    nouts = len(out_names)
    f = jax.jit(
        shard_map(_call, mesh=mesh,
                  in_specs=(PartitionSpec("core"),) * (n_params + nouts),
                  out_specs=(PartitionSpec("core"),) * nouts,
                  check_rep=False),
        keep_unused=True,
    )
    sh = NamedSharding(mesh, PartitionSpec("core"))
    args = [
        jax.device_put(
            np.concatenate([np.asarray(m[name]) for m in in_maps], axis=0), sh)
        for name in in_names
    ] + [
        jax.device_put(
            np.zeros((C * z.shape[0], *z.shape[1:]), z.dtype), sh)
        for z in zero_outs
    ]

    def timed(k):
        # k async back-to-back dispatches; the terminal pipelines them, so
        # the k-slope isolates device execution from tunnel RTT.
        t0 = time.time()
        rs = [f(*args) for _ in range(k)]
        jax.block_until_ready(rs)
        return time.time() - t0

    timed(1)                            # compile + warmup
    timed(reps)
    # The tunnel adds bursty positive noise per dispatch, so the device
    # exec time is the slope of the best-case envelope: min T(reps) over
    # many interleaved samples against the median T(1) dispatch floor (the
    # min T(1) occasionally under-measures and would inflate the slope).
    # In congested windows the whole batch pipelines inside one inflated
    # round trip and the slope degenerates toward 0; detect that (tiny
    # slope or inflated floor) and resample, keeping the largest estimate
    # as a fallback.
    nsamp = 14
    best = 0
    floor0 = None
    for attempt in range(4):
        t1s, tns = [], []
        for _ in range(nsamp):
            t1s.append(timed(1))
            tns.append(timed(reps))
        t1s.sort()
        tns.sort()
        med1, minn = t1s[len(t1s) // 2], tns[0]
        floor0 = min(floor0, t1s[0]) if floor0 is not None else t1s[0]
        est = int(max(minn - min(med1, minn), 0.0) / (reps - 1) * 1e9)
        best = max(best, est)
        clean = med1 <= 1.1 * floor0 and est >= 30_000
        print(f"[kernel] {label}[{attempt}]: medT(1)={med1*1e3:.2f} ms"
              f"  minT({reps})={minn*1e3:.2f} ms  est={est} ns"
              f"{' ok' if clean else ' retry'}", flush=True)
        if clean:
            return est
    return best


def _run(nc, in_maps, label):
    res = run_bass_kernel_spmd(nc, in_maps, list(range(C)))
    reps = int(os.environ.get("GNN_TIME_REPS", "0"))
    if reps > 1:
        _EXEC_TIMES_NS.append(_time_spmd(nc, in_maps, reps, label))
    return res.results


def kernel(x, edge_index, env_edge_attr, act_edge_attr, W0, b0, W1, b1):
    _EXEC_TIMES_NS.clear()

    x = np.asarray(x, np.float32)
    p = _prep(x, edge_index, env_edge_attr, act_edge_attr)

    xg = np.ascontiguousarray(x.astype(mybir.dt.np(BF16)))
    w0t = np.ascontiguousarray(np.asarray(W0, np.float32)[:DIN])
    w0b = np.ascontiguousarray(np.asarray(W0, np.float32)[DIN:])
    b0v = np.asarray(b0, np.float32).reshape(DH, 1)
    w1t = np.ascontiguousarray(np.asarray(W1, np.float32)[:DH])
    w1b = np.ascontiguousarray(np.asarray(W1, np.float32)[DH:])
    b1v = np.asarray(b1, np.float32).reshape(DOUT, 1)

    # ---- layer 0 ----
    nc0 = build_l0(_make_nc(), p)
    in_maps0 = [
        dict(xg=xg, xT=p["xT"][c], pk0=p["pk0"][c],
             iotap=p["iota"], w0t=w0t, w0b=w0b, b0=b0v)
        for c in range(C)
    ]
    res0 = _run(nc0, in_maps0, "L0")

    h = np.empty((N_NODES, DH), np.float32)
    hT_all = np.empty((C, DH, NPAD), np.float32)
    for c in range(C):
        hT_all[c] = res0[c]["hT"]
        h[c * NCORE:(c + 1) * NCORE] = hT_all[c][:, :NCORE].T
    hgb = np.zeros((N_NODES, DIN), mybir.dt.np(BF16))
    hgb[:, :DH] = h

    # ---- layer 1 ----
    nc1 = build_l1(_make_nc(), p)
    in_maps1 = [
        dict(hg=hgb, hTp=hT_all[c], pk1=p["pk1"][c],
             iotap=p["iota"], w1t=w1t, w1b=w1b, b1=b1v)
        for c in range(C)
    ]
    res1 = _run(nc1, in_maps1, "L1")

    out = np.empty((N_NODES, DOUT), np.float32)
    for c in range(C):
        out[c * NCORE:(c + 1) * NCORE] = res1[c]["outT"][:, :NCORE].T
    if _EXEC_TIMES_NS:
        print(f"[kernel] total HW exec time: {sum(_EXEC_TIMES_NS)} ns",
              flush=True)
    return out
